# revision 11
# baseline (speedup 1.0000x reference)
"""Trainium2 Bass kernel for nn_ContrastiveLoss (N=384, D=128, 8 cores).

Sorted-domain prefix-sum formulation (validated vs the reference):
  Sort columns by label value y once (host-side packing).  For row i at
  sorted position m, the contrastive mask sums collapse to interval sums
  of the sorted weight rows:
    U[m,k] = w[m,k]*[ys_k > ys_m],  V[m,k] = w[m,k]*[ys_k <= ys_m][k != m]
    PUex[m,t] = sum_{k<t} U[m,k],   PVex likewise (exclusive prefixes)
    denom[m,p] = (POS_W-1)*PUex[m, t1[m,p]] + T1[m] + NEG_W*PVex[m, t0[m,p]]
  where the rank tables t1/t0 depend only on the (tiny) targets input and
  are precomputed host-side.  The N^3 masked-comparison einsum becomes:
  3 prefix matmuls (PE) + GPSIMD ap_gather lookups + one fused Ln+rowsum.

Per core (48 sorted rows): w-matrix via 3 distance matmuls + DVE/ACT ops,
prefix sums in one PSUM tile [96, 385], D = scaled prefix array [48, 770],
replicated x16 across partitions by DMA so each GPSIMD 16-lane group
gathers one row's 784 indices; Ln(+accumulate) on ACT, final column-sum
matmul.  Host sums the per-core partials.
"""

import os
import sys

import numpy as np

for _p in ("/opt/trn_rl_repo", "/root/.axon_site/_ro/trn_rl_repo"):
    if os.path.isdir(_p) and _p not in sys.path:
        sys.path.insert(0, _p)

import concourse.bass as bass
import concourse.bacc as bacc
import concourse.mybir as mybir
from concourse import tile
from concourse.bass_utils import run_bass_kernel_spmd

F32 = mybir.dt.float32
I16 = mybir.dt.int16
AF = mybir.ActivationFunctionType
OP = mybir.AluOpType

B = 192          # batch
N = 2 * B        # 384 rows/cols
D = 128          # embedding dim
NC = 8           # cores
R = N // NC      # 48 rows per core
CH = N // 128    # 3 chunks of the k dimension
NB = R // 8      # 6 gather blocks of 8 rows
TW = N + 1       # 385 prefix positions t in 0..384
DW = 2 * TW      # 770 = [DPU | DPV]
GW = 784         # gather indices per row (mult of 16): 392 + 392
HGW = GW // 2    # 392 = 384 p-cols + 8 own-copies
IW = GW // 16    # 49 idx columns per block
IWP = 56         # padded idx columns per block (112 B, 16-aligned slices)
DWP = DW + 2     # padded crossin block stride (3088 B, 16-aligned slices)

TEMP = 2.0
TAU = 1.0
POS_W = 0.1
NEG_W = 1.0

# packed fp32 input layout [128, PW]
O_ZT = 0                 # 0:384      zsT (sorted z, transposed)
O_ZOWN = N               # 384:432    zsT own columns
O_YOWN = N + R           # 432:480    ys of own rows (bcast down partitions)
O_IOWN = N + 2 * R       # 480:528    global sorted idx of own rows (f32)
O_YCOL = N + 3 * R       # 528:531    ys per k-chunk column
O_JCOL = O_YCOL + CH     # 531:534    global k idx per chunk column (f32)
O_IOTA = O_JCOL + CH     # 534:919    iota row 0..384 (partition 0)
O_SEL = O_IOTA + TW      # 919:920    sel16 column (1.0 at part%16==0)
PW = O_SEL + 1           # 920
# idx input: int16 [128, NB*IW]


def _build_program():
    nc = bacc.Bacc("TRN2", target_bir_lowering=False, debug=False, num_devices=NC)

    packed = nc.dram_tensor("packed", [128, PW], F32, kind="ExternalInput").ap()
    idxs = nc.dram_tensor("idxs", [128, NB * IWP], I16, kind="ExternalInput").ap()
    out = nc.dram_tensor("out", [2, R], F32, kind="ExternalOutput").ap()

    with tile.TileContext(nc) as tc:
        with (
            tc.tile_pool(name="big", bufs=1) as big,
            tc.tile_pool(name="small", bufs=1) as small,
            tc.tile_pool(name="chunk", bufs=3) as chunk,
            tc.tile_pool(name="ps_a", bufs=2, space="PSUM") as ps_a,
            tc.tile_pool(name="ps_gt", bufs=3, space="PSUM") as ps_gt,
            tc.tile_pool(name="ps_uv", bufs=1, space="PSUM") as ps_uv,
            tc.tile_pool(name="ps_cs", bufs=1, space="PSUM") as ps_cs,
            tc.tile_pool(name="dram", bufs=1, space="DRAM") as dram,
        ):
            # ---------- input DMAs ----------
            pk = big.tile([128, PW], F32, tag="pk")
            nc.sync.dma_start(pk[:], packed)
            zT = pk[:, O_ZT:O_ZT + N]
            zTown = pk[:, O_ZOWN:O_ZOWN + R]
            ysown = pk[:, O_YOWN:O_YOWN + R]
            idxown = pk[:, O_IOWN:O_IOWN + R]
            yscol = pk[:, O_YCOL:O_YCOL + CH]
            jcol = pk[:, O_JCOL:O_JCOL + CH]
            iotarow = pk[0:1, O_IOTA:O_IOTA + TW]
            sel16 = pk[:, O_SEL:O_SEL + 1]

            it0 = big.tile([128, NB * IWP], I16, tag="it0")
            nc.sync.dma_start(it0[:], idxs)
            # route idx through DVE so gathers carry only one DMA-queue wait
            it = big.tile([128, NB * IWP], I16, tag="it")
            nc.vector.tensor_copy(it[:], it0[:])

            ones128 = small.tile([128, 1], F32, tag="ones128")
            nc.vector.memset(ones128[:], 1.0)
            onesrow = small.tile([1, 128], F32, tag="onesrow")
            nc.vector.memset(onesrow[:], 1.0)

            # ---------- squared norms ----------
            zsq = big.tile([128, N], F32, tag="zsq")
            nc.vector.tensor_tensor(zsq[:], zT, zT, op=OP.mult)
            zsqown = small.tile([128, R], F32, tag="zsqown")
            nc.vector.tensor_tensor(zsqown[:], zTown, zTown, op=OP.mult)

            n2own_ps = ps_a.tile([1, R], F32, tag="a")
            nc.tensor.matmul(n2own_ps[:], ones128[:], zsqown[:], start=True, stop=True)
            n2own_s = small.tile([1, R], F32, tag="n2own_s")
            nc.vector.tensor_copy(n2own_s[:], n2own_ps[:])
            n2ownrep_ps = ps_a.tile([128, R], F32, tag="a")
            nc.tensor.matmul(n2ownrep_ps[:], onesrow[:], n2own_s[:], start=True, stop=True)
            n2ownrep = small.tile([128, R], F32, tag="n2ownrep")
            nc.vector.tensor_copy(n2ownrep[:], n2ownrep_ps[:])

            n2colc = small.tile([128, CH], F32, tag="n2colc")
            for c in range(CH):
                n2c_ps = ps_a.tile([128, 1], F32, tag="a")
                nc.tensor.matmul(
                    n2c_ps[:], zsq[:, c * 128:(c + 1) * 128], ones128[:],
                    start=True, stop=True,
                )
                nc.vector.tensor_copy(n2colc[:, c:c + 1], n2c_ps[:])

            # ---------- Texc: [k < t] per chunk ----------
            trep_ps = ps_a.tile([128, TW], F32, tag="a")
            nc.tensor.matmul(trep_ps[:], onesrow[:], iotarow, start=True, stop=True)
            trep = big.tile([128, TW], F32, tag="trep")
            nc.vector.tensor_copy(trep[:], trep_ps[:])
            texc = big.tile([128, CH * TW], F32, tag="texc")
            for c in range(CH):
                nc.vector.tensor_scalar(
                    texc[:, c * TW:(c + 1) * TW], trep[:], jcol[:, c:c + 1], None,
                    op0=OP.is_gt,
                )

            # ---------- stage A: w matrix (transposed) per chunk ----------
            UW = 2 * R  # per-chunk lhsT cols: U(48) | V(48)
            uvt = big.tile([128, CH * UW], F32, tag="uvt")
            cs_ps = ps_cs.tile([1, R], F32, tag="cs")
            for c in range(CH):
                ycolbc = yscol[:, c:c + 1].to_broadcast((128, R))
                samet = chunk.tile([128, R], F32, tag="samet")
                nc.vector.tensor_tensor(samet[:], ysown, ycolbc, op=OP.is_lt)
                ndt = chunk.tile([128, R], F32, tag="ndt")
                nc.vector.tensor_tensor(
                    ndt[:], idxown, jcol[:, c:c + 1].to_broadcast((128, R)),
                    op=OP.not_equal,
                )
                gt_ps = ps_gt.tile([128, R], F32, tag="gt")
                nc.tensor.matmul(
                    gt_ps[:], zT[:, c * 128:(c + 1) * 128], zTown,
                    start=True, stop=True,
                )
                sqt = chunk.tile([128, R], F32, tag="sqt")
                nc.vector.scalar_tensor_tensor(
                    sqt[:], gt_ps[:], -2.0, n2ownrep[:], op0=OP.mult, op1=OP.add
                )
                sqr = chunk.tile([128, R], F32, tag="sqr")
                nc.scalar.activation(sqr[:], sqt[:], AF.Relu, bias=n2colc[:, c:c + 1])
                distt = chunk.tile([128, R], F32, tag="distt")
                nc.scalar.activation(distt[:], sqr[:], AF.Sqrt)
                et = chunk.tile([128, R], F32, tag="et")
                nc.scalar.activation(et[:], distt[:], AF.Exp, scale=-1.0 / TEMP)
                atcraw = chunk.tile([128, R], F32, tag="atcraw")
                nc.vector.tensor_tensor(atcraw[:], ysown, ycolbc, op=OP.subtract)
                atc = chunk.tile([128, R], F32, tag="atc")
                nc.scalar.activation(atc[:], atcraw[:], AF.Abs)
                dwt = chunk.tile([128, R], F32, tag="dwt")
                nc.scalar.activation(dwt[:], atc[:], AF.Sigmoid, scale=TAU)
                wt = chunk.tile([128, R], F32, tag="wt")
                nc.vector.tensor_tensor(wt[:], et[:], dwt[:], op=OP.mult)
                # U / V columns for the prefix matmul lhsT
                nc.vector.tensor_tensor(
                    uvt[:, c * UW:c * UW + R], wt[:], samet[:], op=OP.mult
                )
                vm = chunk.tile([128, R], F32, tag="vm")
                nc.vector.tensor_tensor(vm[:], ndt[:], samet[:], op=OP.subtract)
                nc.vector.tensor_tensor(
                    uvt[:, c * UW + R:c * UW + 2 * R], wt[:], vm[:], op=OP.mult
                )
                # off-diagonal dist row-sums (for the s term)
                wdist = chunk.tile([128, R], F32, tag="wdist")
                nc.vector.tensor_tensor(wdist[:], distt[:], ndt[:], op=OP.mult)
                nc.tensor.matmul(
                    cs_ps[:], ones128[:], wdist[:], start=(c == 0), stop=(c == CH - 1)
                )

            # ---------- prefix sums: PUex and PVex, both at partitions 0..47 ----------
            pu_ps = ps_uv.tile([R, TW], F32, tag="pu")
            pv_ps = ps_uv.tile([R, TW], F32, tag="pv")
            for c in range(CH):
                nc.tensor.matmul(
                    pu_ps[:], uvt[:, c * UW:c * UW + R],
                    texc[:, c * TW:(c + 1) * TW],
                    start=(c == 0), stop=(c == CH - 1),
                )
            for c in range(CH):
                nc.tensor.matmul(
                    pv_ps[:], uvt[:, c * UW + R:(c + 1) * UW],
                    texc[:, c * TW:(c + 1) * TW],
                    start=(c == 0), stop=(c == CH - 1),
                )

            # ---------- D = [(POS_W-1)*PUex + T1 | NEG_W*PVex]  [48, 770] ----------
            t1sb = small.tile([R, 1], F32, tag="t1sb")
            nc.vector.tensor_copy(t1sb[:], pu_ps[:, N:N + 1])
            darr = big.tile([R, DW], F32, tag="darr")
            nc.vector.scalar_tensor_tensor(
                darr[:, 0:TW], pu_ps[:], POS_W - 1.0,
                t1sb[:].to_broadcast((R, TW)), op0=OP.mult, op1=OP.add,
            )
            if NEG_W == 1.0:
                nc.vector.tensor_copy(darr[:, TW:DW], pv_ps[:])
            else:
                nc.vector.tensor_scalar(
                    darr[:, TW:DW], pv_ps[:], NEG_W, None, op0=OP.mult
                )

            # ---------- per block: replicate x16 via DRAM (write-side bcast), gather ----------
            # A single write DMA materializes 16 copies of each row (free-dim
            # 0-stride on the SBUF read side), a single plain read brings the
            # [128, DW] replicated block back: every stage has one dependency,
            # so tile's one-queue-sem-per-instruction sync stays correct.
            dD = dram.tile([NB * 128, DW], F32, tag="dD")
            crossin = big.tile([128, NB * DWP], F32, tag="crossin")
            gout = big.tile([128, NB * GW], F32, tag="gout")
            rowtots = small.tile([128, NB], F32, tag="rowtots")
            for b in range(NB):
                nc.sync.dma_start(
                    dD[b * 128:(b + 1) * 128, :].rearrange("(r g) f -> r g f", g=16),
                    darr[b * 8:(b + 1) * 8, :].unsqueeze(1).to_broadcast((8, 16, DW)),
                )
                cin_b = crossin[:, b * DWP:b * DWP + DW]
                nc.sync.dma_start(cin_b, dD[b * 128:(b + 1) * 128, :])
                go_b = gout[:, b * GW:(b + 1) * GW]
                nc.gpsimd.ap_gather(
                    go_b, cin_b, it[:, b * IWP:b * IWP + IW],
                    channels=128, num_elems=DW, d=1, num_idxs=GW,
                )
                dn = chunk.tile([128, HGW], F32, tag="dn")
                nc.vector.tensor_tensor(
                    dn[:], go_b[:, 0:HGW], go_b[:, HGW:GW], op=OP.add
                )
                lnt = chunk.tile([128, HGW], F32, tag="lnt")
                acc = chunk.tile([128, 1], F32, tag="acc")
                nc.scalar.activation(lnt[:], dn[:], AF.Ln, accum_out=acc[:])
                # accum counts own-col once in 0..383 plus 8 pad copies -> -9x
                nc.vector.scalar_tensor_tensor(
                    rowtots[:, b:b + 1], lnt[:, N:N + 1], -9.0, acc[:],
                    op0=OP.mult, op1=OP.add,
                )

            # ---------- final reduction ----------
            lnacc_ps = ps_a.tile([1, NB], F32, tag="a")
            nc.tensor.matmul(lnacc_ps[:], sel16, rowtots[:], start=True, stop=True)
            outrow = small.tile([1, R + NB], F32, tag="outrow")
            nc.vector.tensor_copy(outrow[0:1, 0:R], cs_ps[:])
            nc.vector.tensor_copy(outrow[0:1, R:R + NB], lnacc_ps[:])
            nc.sync.dma_start(out[0:1, :], outrow[0:1, 0:R])
            nc.sync.dma_start(out[1:2, 0:NB], outrow[0:1, R:R + NB])

    nc.compile()
    return nc


_NC_CACHE = None


def _get_nc():
    global _NC_CACHE
    if _NC_CACHE is None:
        _NC_CACHE = _build_program()
    return _NC_CACHE


def _host_prep(embeddings, targets):
    emb = np.ascontiguousarray(np.asarray(embeddings, dtype=np.float32))
    tgt = np.ascontiguousarray(np.asarray(targets, dtype=np.float32))
    z = emb.transpose(1, 0, 2).reshape(N, D)
    y = np.concatenate([tgt, tgt], axis=0)[:, 0]

    order = np.argsort(y, kind="stable")
    ys = y[order]
    zs = z[order]
    zsT = np.ascontiguousarray(zs.T)  # [D, N]

    # rank tables (depend only on targets)
    A = np.abs(ys[None, :] - ys[:, None]).astype(np.float32)
    hi = np.searchsorted(ys, ys, side="right") - 1
    t1 = np.empty((N, N), np.int32)
    t0 = np.empty((N, N), np.int32)
    for m in range(N):
        h = hi[m]
        q1 = np.searchsorted(A[m, h + 1:], A[m], side="left")
        q0 = np.searchsorted(A[m, :h + 1][::-1], A[m], side="left")
        t1[m] = h + 1 + q1
        t0[m] = h + 1 - q0

    jidx = np.arange(N, dtype=np.float32)
    iota = np.arange(TW, dtype=np.float32)
    sel = (np.arange(128) % 16 == 0).astype(np.float32)

    in_maps = []
    for core in range(NC):
        sl = slice(core * R, (core + 1) * R)
        p = np.zeros((128, PW), np.float32)
        p[:, O_ZT:O_ZT + N] = zsT
        p[:, O_ZOWN:O_ZOWN + R] = zsT[:, sl]
        p[:, O_YOWN:O_YOWN + R] = ys[None, sl]
        p[:, O_IOWN:O_IOWN + R] = jidx[None, sl]
        p[:, O_YCOL:O_YCOL + CH] = ys.reshape(CH, 128).T
        p[:, O_JCOL:O_JCOL + CH] = jidx.reshape(CH, 128).T
        p[0, O_IOTA:O_IOTA + TW] = iota
        p[:, O_SEL] = sel

        # gather index tile: per block b (8 rows), wrapped 16-partition layout
        it = np.zeros((128, NB * IWP), np.int16)
        for bidx in range(NB):
            rows = core * R + bidx * 8 + np.arange(8)
            q = np.empty((8, GW), np.int16)
            q[:, 0:N] = t1[rows]
            q[:, N:HGW] = (hi[rows] + 1)[:, None]
            q[:, HGW:HGW + N] = TW + t0[rows]
            q[:, HGW + N:GW] = (TW + hi[rows] + 1)[:, None]
            # position j of group g -> it[16g + j%16, IW*b + j//16]
            wrapped = q.reshape(8, IW, 16).transpose(0, 2, 1)  # [g, j%16, j//16]
            it[:, bidx * IWP:bidx * IWP + IW] = wrapped.reshape(128, IW)
        in_maps.append({"packed": p, "idxs": it})
    return in_maps


def _reduce_outs(outs_list):
    tot_dist = 0.0
    tot_logd = 0.0
    for o in outs_list:
        o = np.asarray(o, dtype=np.float64)
        tot_dist += o[0, :].sum()
        tot_logd += o[1, 0:NB].sum()
    s_total = -tot_dist / TEMP
    loss = -(s_total - tot_logd) / (N * (N - 1))
    return np.float32(loss)


def _run(embeddings, targets, trace=False, **kw):
    nc = _get_nc()
    in_maps = _host_prep(embeddings, targets)
    res = run_bass_kernel_spmd(nc, in_maps, list(range(NC)), trace=trace, **kw)
    outs = [res.results[c]["out"] for c in range(NC)]
    return _reduce_outs(outs), res


def kernel(embeddings, targets):
    loss, _ = _run(embeddings, targets, trace=False)
    return loss


# revision 13
# speedup vs baseline: 1.3337x; 1.3337x over previous
"""Trainium2 Bass kernel for nn_ContrastiveLoss (N=384, D=128, 8 cores).

Sorted-domain prefix-sum formulation (validated vs the reference):
  Sort columns by label value y once (host-side packing).  For row i at
  sorted position m, the contrastive mask sums collapse to interval sums
  of the sorted weight rows:
    U[m,k] = w[m,k]*[ys_k > ys_m],  V[m,k] = w[m,k]*[ys_k <= ys_m][k != m]
    PUex[m,t] = sum_{k<t} U[m,k],   PVex likewise (exclusive prefixes)
    denom[m,p] = (POS_W-1)*PUex[m, t1[m,p]] + T1[m] + NEG_W*PVex[m, t0[m,p]]
  where the rank tables t1/t0 depend only on the (tiny) targets input and
  are precomputed host-side.  The N^3 masked-comparison einsum becomes:
  3 prefix matmuls (PE) + GPSIMD ap_gather lookups + one fused Ln+rowsum.

Per core (48 sorted rows): w-matrix via 3 distance matmuls + DVE/ACT ops,
prefix sums in one PSUM tile [96, 385], D = scaled prefix array [48, 770],
replicated x16 across partitions by DMA so each GPSIMD 16-lane group
gathers one row's 784 indices; Ln(+accumulate) on ACT, final column-sum
matmul.  Host sums the per-core partials.
"""

import os
import sys

import numpy as np

for _p in ("/opt/trn_rl_repo", "/root/.axon_site/_ro/trn_rl_repo"):
    if os.path.isdir(_p) and _p not in sys.path:
        sys.path.insert(0, _p)

import concourse.bass as bass
import concourse.bacc as bacc
import concourse.mybir as mybir
from concourse import tile
from concourse.bass_utils import run_bass_kernel_spmd

F32 = mybir.dt.float32
BF16 = mybir.dt.bfloat16
I16 = mybir.dt.int16
AF = mybir.ActivationFunctionType
OP = mybir.AluOpType

B = 192          # batch
N = 2 * B        # 384 rows/cols
D = 128          # embedding dim
NC = 8           # cores
R = N // NC      # 48 rows per core
CH = N // 128    # 3 chunks of the k dimension
NB = R // 8      # 6 gather blocks of 8 rows
TW = N + 1       # 385 prefix positions t in 0..384
DW = 2 * TW      # 770 = [DPU | DPV]
GW = 784         # gather indices per row (mult of 16): 392 + 392
HGW = GW // 2    # 392 = 384 p-cols + 8 own-copies
IW = GW // 16    # 49 idx columns per block
IWP = 56         # padded idx columns per block (112 B, 16-aligned slices)
DWP = DW + 2     # padded crossin block stride (3088 B, 16-aligned slices)

TEMP = 2.0
TAU = 1.0
POS_W = 0.1
NEG_W = 1.0

# packed fp32 input layout [128, PW]
O_ZT = 0                 # 0:384      zsT (sorted z, transposed)
O_ZOWN = N               # 384:432    zsT own columns
O_YOWN = N + R           # 432:480    ys of own rows (bcast down partitions)
O_IOWN = N + 2 * R       # 480:528    global sorted idx of own rows (f32)
O_YCOL = N + 3 * R       # 528:531    ys per k-chunk column
O_JCOL = O_YCOL + CH     # 531:534    global k idx per chunk column (f32)
O_IOTA = O_JCOL + CH     # 534:919    iota row 0..384 (partition 0)
O_SEL = O_IOTA + TW      # 919:920    sel16 column (1.0 at part%16==0)
O_EB = O_SEL + 1         # 920:1304   E_b selection lhsT, bf16-pairs in fp32
PW = O_EB + NB * 64      # 1304
# idx input: int16 [128, NB*IW]


def _build_program():
    nc = bacc.Bacc("TRN2", target_bir_lowering=False, debug=False, num_devices=NC)

    packed = nc.dram_tensor("packed", [128, PW], F32, kind="ExternalInput").ap()
    idxs = nc.dram_tensor("idxs", [128, NB * IWP], I16, kind="ExternalInput").ap()
    out = nc.dram_tensor("out", [2, R], F32, kind="ExternalOutput").ap()

    with tile.TileContext(nc) as tc:
        with (
            tc.tile_pool(name="big", bufs=1) as big,
            tc.tile_pool(name="small", bufs=1) as small,
            tc.tile_pool(name="chunk", bufs=3) as chunk,
            tc.tile_pool(name="ps_a", bufs=1, space="PSUM") as ps_a,
            tc.tile_pool(name="ps_gt", bufs=2, space="PSUM") as ps_gt,
            tc.tile_pool(name="ps_uv", bufs=1, space="PSUM") as ps_uv,
            tc.tile_pool(name="ps_cs", bufs=1, space="PSUM") as ps_cs,
            tc.tile_pool(name="cpool", bufs=3) as cpool,
            tc.tile_pool(name="ps_rep", bufs=2, space="PSUM") as ps_rep,
        ):
            # ---------- input DMAs ----------
            pk = big.tile([128, PW], F32, tag="pk")
            nc.sync.dma_start(pk[:], packed)
            zT = pk[:, O_ZT:O_ZT + N]
            zTown = pk[:, O_ZOWN:O_ZOWN + R]
            ysown = pk[:, O_YOWN:O_YOWN + R]
            idxown = pk[:, O_IOWN:O_IOWN + R]
            yscol = pk[:, O_YCOL:O_YCOL + CH]
            jcol = pk[:, O_JCOL:O_JCOL + CH]
            iotarow = pk[0:1, O_IOTA:O_IOTA + TW]
            sel16 = pk[:, O_SEL:O_SEL + 1]

            it0 = big.tile([128, NB * IWP], I16, tag="it0")
            nc.sync.dma_start(it0[:], idxs)
            # route idx through DVE so gathers carry only one DMA-queue wait
            it = big.tile([128, NB * IWP], I16, tag="it")
            nc.vector.tensor_copy(it[:], it0[:])

            ones128 = small.tile([128, 1], F32, tag="ones128")
            nc.vector.memset(ones128[:], 1.0)
            onesrow = small.tile([1, 128], F32, tag="onesrow")
            nc.vector.memset(onesrow[:], 1.0)

            # ---------- squared norms ----------
            zsq = big.tile([128, N], F32, tag="zsq")
            nc.vector.tensor_tensor(zsq[:], zT, zT, op=OP.mult)
            zsqown = small.tile([128, R], F32, tag="zsqown")
            nc.vector.tensor_tensor(zsqown[:], zTown, zTown, op=OP.mult)

            n2own_ps = ps_a.tile([1, R], F32, tag="a")
            nc.tensor.matmul(n2own_ps[:], ones128[:], zsqown[:], start=True, stop=True)
            n2own_s = small.tile([1, R], F32, tag="n2own_s")
            nc.vector.tensor_copy(n2own_s[:], n2own_ps[:])
            n2ownrep_ps = ps_a.tile([128, R], F32, tag="a")
            nc.tensor.matmul(n2ownrep_ps[:], onesrow[:], n2own_s[:], start=True, stop=True)
            n2ownrep = small.tile([128, R], F32, tag="n2ownrep")
            nc.vector.tensor_copy(n2ownrep[:], n2ownrep_ps[:])

            n2colc = small.tile([128, CH], F32, tag="n2colc")
            for c in range(CH):
                n2c_ps = ps_a.tile([128, 1], F32, tag="a")
                nc.tensor.matmul(
                    n2c_ps[:], zsq[:, c * 128:(c + 1) * 128], ones128[:],
                    start=True, stop=True,
                )
                nc.vector.tensor_copy(n2colc[:, c:c + 1], n2c_ps[:])

            # ---------- Texc: [k < t] per chunk ----------
            trep_ps = ps_a.tile([128, TW], F32, tag="a")
            nc.tensor.matmul(trep_ps[:], onesrow[:], iotarow, start=True, stop=True)
            trep = big.tile([128, TW], F32, tag="trep")
            nc.vector.tensor_copy(trep[:], trep_ps[:])
            texc = big.tile([128, CH * TW], BF16, tag="texc")
            for c in range(CH):
                nc.vector.tensor_scalar(
                    texc[:, c * TW:(c + 1) * TW], trep[:], jcol[:, c:c + 1], None,
                    op0=OP.is_gt,
                )

            # ---------- stage A: w matrix (transposed) per chunk ----------
            UW = 2 * R  # per-chunk lhsT cols: U(48) | V(48)
            uvt = big.tile([128, CH * UW], BF16, tag="uvt")
            cs_ps = ps_cs.tile([1, R], F32, tag="cs")
            for c in range(CH):
                ycolbc = yscol[:, c:c + 1].to_broadcast((128, R))
                samet = chunk.tile([128, R], F32, tag="samet")
                nc.vector.tensor_tensor(samet[:], ysown, ycolbc, op=OP.is_lt)
                ndt = chunk.tile([128, R], F32, tag="ndt")
                nc.vector.tensor_tensor(
                    ndt[:], idxown, jcol[:, c:c + 1].to_broadcast((128, R)),
                    op=OP.not_equal,
                )
                gt_ps = ps_gt.tile([128, R], F32, tag="gt")
                nc.tensor.matmul(
                    gt_ps[:], zT[:, c * 128:(c + 1) * 128], zTown,
                    start=True, stop=True,
                )
                sqt = chunk.tile([128, R], F32, tag="sqt")
                nc.vector.scalar_tensor_tensor(
                    sqt[:], gt_ps[:], -2.0, n2ownrep[:], op0=OP.mult, op1=OP.add
                )
                sqr = chunk.tile([128, R], F32, tag="sqr")
                nc.scalar.activation(sqr[:], sqt[:], AF.Relu, bias=n2colc[:, c:c + 1])
                distt = chunk.tile([128, R], F32, tag="distt")
                nc.scalar.activation(distt[:], sqr[:], AF.Sqrt)
                et = chunk.tile([128, R], F32, tag="et")
                nc.scalar.activation(et[:], distt[:], AF.Exp, scale=-1.0 / TEMP)
                atcraw = chunk.tile([128, R], F32, tag="atcraw")
                nc.vector.tensor_tensor(atcraw[:], ysown, ycolbc, op=OP.subtract)
                atc = chunk.tile([128, R], F32, tag="atc")
                nc.scalar.activation(atc[:], atcraw[:], AF.Abs)
                dwt = chunk.tile([128, R], F32, tag="dwt")
                nc.scalar.activation(dwt[:], atc[:], AF.Sigmoid, scale=TAU)
                wt = chunk.tile([128, R], F32, tag="wt")
                nc.vector.tensor_tensor(wt[:], et[:], dwt[:], op=OP.mult)
                # U / V columns for the prefix matmul lhsT
                nc.vector.tensor_tensor(
                    uvt[:, c * UW:c * UW + R], wt[:], samet[:], op=OP.mult
                )
                vm = chunk.tile([128, R], F32, tag="vm")
                nc.vector.tensor_tensor(vm[:], ndt[:], samet[:], op=OP.subtract)
                nc.vector.tensor_tensor(
                    uvt[:, c * UW + R:c * UW + 2 * R], wt[:], vm[:], op=OP.mult
                )
                # off-diagonal dist row-sums (for the s term)
                wdist = chunk.tile([128, R], F32, tag="wdist")
                nc.vector.tensor_tensor(wdist[:], distt[:], ndt[:], op=OP.mult)
                nc.tensor.matmul(
                    cs_ps[:], ones128[:], wdist[:], start=(c == 0), stop=(c == CH - 1)
                )

            # ---------- prefix sums: PUex and PVex, both at partitions 0..47 ----------
            pu_ps = ps_uv.tile([R, TW], F32, tag="pu")
            pv_ps = ps_uv.tile([R, TW], F32, tag="pv")
            for c in range(CH):
                nc.tensor.matmul(
                    pu_ps[:], uvt[:, c * UW:c * UW + R],
                    texc[:, c * TW:(c + 1) * TW],
                    start=(c == 0), stop=(c == CH - 1),
                )
            for c in range(CH):
                nc.tensor.matmul(
                    pv_ps[:], uvt[:, c * UW + R:(c + 1) * UW],
                    texc[:, c * TW:(c + 1) * TW],
                    start=(c == 0), stop=(c == CH - 1),
                )

            # ---------- D halves (bf16): DPU = (POS_W-1)*PUex + T1, DPV = NEG_W*PVex ----------
            t1sb = small.tile([R, 1], F32, tag="t1sb")
            nc.vector.tensor_copy(t1sb[:], pu_ps[:, N:N + 1])
            darrA = small.tile([R, TW], BF16, tag="darrA")
            nc.vector.scalar_tensor_tensor(
                darrA[:], pu_ps[:], POS_W - 1.0,
                t1sb[:].to_broadcast((R, TW)), op0=OP.mult, op1=OP.add,
            )
            darrB = small.tile([R, TW], BF16, tag="darrB")
            if NEG_W == 1.0:
                nc.vector.tensor_copy(darrB[:], pv_ps[:])
            else:
                nc.vector.tensor_scalar(darrB[:], pv_ps[:], NEG_W, None, op0=OP.mult)

            # ---------- per block: replicate x16 on the PE, copy, gather ----------
            # Row r -> partitions 16r..16r+15 via a bf16 selection matmul
            # (E_b[k, c] = [k == 8b + c//16]); PSUM halves are copied to one
            # SBUF tile (DVE + ACT split) that the GPSIMD gather reads.  All
            # block-loop dependencies are engine semaphores - no DMA queues.
            gout = big.tile([128, NB * GW], F32, tag="gout")
            rowtots = small.tile([128, NB], F32, tag="rowtots")
            for b in range(NB):
                ebs = pk[0:R, O_EB + b * 64:O_EB + (b + 1) * 64].bitcast(BF16)
                repA_ps = ps_rep.tile([128, TW], F32, tag="rep")
                nc.tensor.matmul(repA_ps[:], ebs, darrA[:], start=True, stop=True)
                repB_ps = ps_rep.tile([128, TW], F32, tag="rep")
                nc.tensor.matmul(repB_ps[:], ebs, darrB[:], start=True, stop=True)
                cin_b = cpool.tile([128, 784], F32, tag="cin")
                nc.vector.tensor_copy(cin_b[:, 0:TW], repA_ps[:])
                nc.scalar.activation(cin_b[:, TW:DW], repB_ps[:], AF.Copy)
                go_b = gout[:, b * GW:(b + 1) * GW]
                nc.gpsimd.ap_gather(
                    go_b, cin_b[:, 0:DW], it[:, b * IWP:b * IWP + IW],
                    channels=128, num_elems=DW, d=1, num_idxs=GW,
                )
                dn = chunk.tile([128, HGW], F32, tag="dn")
                nc.vector.tensor_tensor(
                    dn[:], go_b[:, 0:HGW], go_b[:, HGW:GW], op=OP.add
                )
                lnt = chunk.tile([128, HGW], F32, tag="lnt")
                acc = chunk.tile([128, 1], F32, tag="acc")
                nc.scalar.activation(lnt[:], dn[:], AF.Ln, accum_out=acc[:])
                # accum counts own-col once in 0..383 plus 8 pad copies -> -9x
                nc.vector.scalar_tensor_tensor(
                    rowtots[:, b:b + 1], lnt[:, N:N + 1], -9.0, acc[:],
                    op0=OP.mult, op1=OP.add,
                )

            # ---------- final reduction ----------
            lnacc_ps = ps_a.tile([1, NB], F32, tag="a")
            nc.tensor.matmul(lnacc_ps[:], sel16, rowtots[:], start=True, stop=True)
            outrow = small.tile([1, R + NB], F32, tag="outrow")
            nc.vector.tensor_copy(outrow[0:1, 0:R], cs_ps[:])
            nc.vector.tensor_copy(outrow[0:1, R:R + NB], lnacc_ps[:])
            nc.sync.dma_start(out[0:1, :], outrow[0:1, 0:R])
            nc.sync.dma_start(out[1:2, 0:NB], outrow[0:1, R:R + NB])

    nc.compile()
    return nc


_NC_CACHE = None


def _get_nc():
    global _NC_CACHE
    if _NC_CACHE is None:
        _NC_CACHE = _build_program()
    return _NC_CACHE


def _host_prep(embeddings, targets):
    emb = np.ascontiguousarray(np.asarray(embeddings, dtype=np.float32))
    tgt = np.ascontiguousarray(np.asarray(targets, dtype=np.float32))
    z = emb.transpose(1, 0, 2).reshape(N, D)
    y = np.concatenate([tgt, tgt], axis=0)[:, 0]

    order = np.argsort(y, kind="stable")
    ys = y[order]
    zs = z[order]
    zsT = np.ascontiguousarray(zs.T)  # [D, N]

    # rank tables (depend only on targets)
    A = np.abs(ys[None, :] - ys[:, None]).astype(np.float32)
    hi = np.searchsorted(ys, ys, side="right") - 1
    t1 = np.empty((N, N), np.int32)
    t0 = np.empty((N, N), np.int32)
    for m in range(N):
        h = hi[m]
        q1 = np.searchsorted(A[m, h + 1:], A[m], side="left")
        q0 = np.searchsorted(A[m, :h + 1][::-1], A[m], side="left")
        t1[m] = h + 1 + q1
        t0[m] = h + 1 - q0

    jidx = np.arange(N, dtype=np.float32)
    iota = np.arange(TW, dtype=np.float32)
    sel = (np.arange(128) % 16 == 0).astype(np.float32)
    import ml_dtypes
    ebpack = np.zeros((R, NB * 64), np.float32)
    for bidx in range(NB):
        E = np.zeros((R, 128), np.float32)
        for c in range(128):
            E[8 * bidx + c // 16, c] = 1.0
        ebpack[:, bidx * 64:(bidx + 1) * 64] = (
            E.astype(ml_dtypes.bfloat16).view(np.float32)
        )

    in_maps = []
    for core in range(NC):
        sl = slice(core * R, (core + 1) * R)
        p = np.zeros((128, PW), np.float32)
        p[:, O_ZT:O_ZT + N] = zsT
        p[:, O_ZOWN:O_ZOWN + R] = zsT[:, sl]
        p[:, O_YOWN:O_YOWN + R] = ys[None, sl]
        p[:, O_IOWN:O_IOWN + R] = jidx[None, sl]
        p[:, O_YCOL:O_YCOL + CH] = ys.reshape(CH, 128).T
        p[:, O_JCOL:O_JCOL + CH] = jidx.reshape(CH, 128).T
        p[0, O_IOTA:O_IOTA + TW] = iota
        p[:, O_SEL] = sel
        p[0:R, O_EB:O_EB + NB * 64] = ebpack

        # gather index tile: per block b (8 rows), wrapped 16-partition layout
        it = np.zeros((128, NB * IWP), np.int16)
        for bidx in range(NB):
            rows = core * R + bidx * 8 + np.arange(8)
            q = np.empty((8, GW), np.int16)
            q[:, 0:N] = t1[rows]
            q[:, N:HGW] = (hi[rows] + 1)[:, None]
            q[:, HGW:HGW + N] = TW + t0[rows]
            q[:, HGW + N:GW] = (TW + hi[rows] + 1)[:, None]
            # position j of group g -> it[16g + j%16, IW*b + j//16]
            wrapped = q.reshape(8, IW, 16).transpose(0, 2, 1)  # [g, j%16, j//16]
            it[:, bidx * IWP:bidx * IWP + IW] = wrapped.reshape(128, IW)
        in_maps.append({"packed": p, "idxs": it})
    return in_maps


def _reduce_outs(outs_list):
    tot_dist = 0.0
    tot_logd = 0.0
    for o in outs_list:
        o = np.asarray(o, dtype=np.float64)
        tot_dist += o[0, :].sum()
        tot_logd += o[1, 0:NB].sum()
    s_total = -tot_dist / TEMP
    loss = -(s_total - tot_logd) / (N * (N - 1))
    return np.float32(loss)


def _run(embeddings, targets, trace=False, **kw):
    nc = _get_nc()
    in_maps = _host_prep(embeddings, targets)
    res = run_bass_kernel_spmd(nc, in_maps, list(range(NC)), trace=trace, **kw)
    outs = [res.results[c]["out"] for c in range(NC)]
    return _reduce_outs(outs), res


def kernel(embeddings, targets):
    loss, _ = _run(embeddings, targets, trace=False)
    return loss


# revision 17
# speedup vs baseline: 2.0277x; 1.5204x over previous
"""Trainium2 Bass kernel for nn_ContrastiveLoss (N=384, D=128, 8 cores).

Sorted-domain formulation (validated vs the reference): sort columns by
label y once (host packing).  With U[i,k] = w[i,k][ys_k > ys_i], V[i,k] =
w[i,k][ys_k <= ys_i][k != i] and exclusive prefixes PUex/PVex:
  p above i: denom = T1 + (POS_W-1)*PUex[i,lo_p] + NEG_W*PVex[i,t0[i,p]]
  p below i: denom = T1 + (POS_W-1)*PUex[i,t1[i,p]] + NEG_W*PVex[i,hi_p+1]
The shared-index (diag) halves are matmuls with constant 0/1 rhs
([k < lo_p], [k <= hi_p]); the per-row (cross) halves are one lookup per
(i,p) done by a single GPSIMD ap_gather (which costs ~27ns/index per
16-partition group - hence halving its index count via the diag matmuls
and issuing exactly one gather instruction).  Rank tables t1/t0 and the
above-mask depend only on the targets and are precomputed host-side.
"""

import os
import sys

import numpy as np

for _p in ("/opt/trn_rl_repo", "/root/.axon_site/_ro/trn_rl_repo"):
    if os.path.isdir(_p) and _p not in sys.path:
        sys.path.insert(0, _p)

import concourse.bass as bass
import concourse.bacc as bacc
import concourse.mybir as mybir
from concourse import tile
from concourse.bass_utils import run_bass_kernel_spmd

F32 = mybir.dt.float32
BF16 = mybir.dt.bfloat16
I16 = mybir.dt.int16
AF = mybir.ActivationFunctionType
OP = mybir.AluOpType

B = 192          # batch
N = 2 * B        # 384 rows/cols
D = 128          # embedding dim
NC = 8           # cores
R = N // NC      # 48 rows per core
CH = N // 128    # 3 chunks of the k dimension
NB = R // 8      # 6 blocks of 8 rows (one row per GPSIMD core group)
TW = N + 1       # 385 prefix positions
DW = 2 * TW      # 770 = [DPU | DPV] per block section of the gather input
GW = N           # 384 cross-gather indices per row
IW = GW // 16    # 24 wrapped idx columns per block

TEMP = 2.0
TAU = 1.0
POS_W = 0.1
NEG_W = 1.0

# packed fp32 input layout [128, PW]
O_ZT = 0                  # zsT (sorted z, transposed) [128, 384]
O_ZOWN = N                # own z columns [128, 48]
O_YOWN = N + R            # ys of own rows bcast down partitions [128, 48]
O_IOWN = N + 2 * R        # global sorted idx of own rows [128, 48]
O_YCOL = N + 3 * R        # ys per k-chunk column [128, 3]
O_JCOL = O_YCOL + CH      # global k idx per chunk column [128, 3]
O_IOTA = O_JCOL + CH      # iota row 0..384 (partition 0) [1, 385]
O_SEL = O_IOTA + TW       # sel16 column (1.0 at part%16==0) [128, 1]
O_EB = O_SEL + 1          # E_b selection lhsT, bf16-pairs in f32 [48, NB*64]
O_LO = O_EB + NB * 64     # lo_p row (partition 0) [1, 384]
O_HI1 = O_LO + N          # hi_p+1 row (partition 0) [1, 384]
O_AB = O_HI1 + N          # above-mask [48, 384] (rows 0..47)
PW = O_AB + N


def _build_program():
    nc = bacc.Bacc("TRN2", target_bir_lowering=False, debug=False, num_devices=NC)

    packed = nc.dram_tensor("packed", [128, PW], F32, kind="ExternalInput").ap()
    idxs = nc.dram_tensor("idxs", [128, NB * IW], I16, kind="ExternalInput").ap()
    out = nc.dram_tensor("out", [2, R], F32, kind="ExternalOutput").ap()

    with tile.TileContext(nc) as tc:
        with (
            tc.tile_pool(name="big", bufs=1) as big,
            tc.tile_pool(name="small", bufs=1) as small,
            tc.tile_pool(name="chunk", bufs=1) as chunk,
            tc.tile_pool(name="ps_a", bufs=1, space="PSUM") as ps_a,
            tc.tile_pool(name="ps_gt", bufs=2, space="PSUM") as ps_gt,
            tc.tile_pool(name="ps_uv", bufs=1, space="PSUM") as ps_uv,
            tc.tile_pool(name="ps_cs", bufs=1, space="PSUM") as ps_cs,
            tc.tile_pool(name="ps_rep", bufs=2, space="PSUM") as ps_rep,
        ):
            # ---------- input DMAs ----------
            pk = big.tile([128, PW], F32, tag="pk")
            nc.sync.dma_start(pk[:], packed)
            zT = pk[:, O_ZT:O_ZT + N]
            zTown = pk[:, O_ZOWN:O_ZOWN + R]
            ysown = pk[:, O_YOWN:O_YOWN + R]
            idxown = pk[:, O_IOWN:O_IOWN + R]
            yscol = pk[:, O_YCOL:O_YCOL + CH]
            jcol = pk[:, O_JCOL:O_JCOL + CH]
            iotarow = pk[0:1, O_IOTA:O_IOTA + TW]
            sel16 = pk[:, O_SEL:O_SEL + 1]
            lorow = pk[0:1, O_LO:O_LO + N]
            hi1row = pk[0:1, O_HI1:O_HI1 + N]
            abmask = pk[0:R, O_AB:O_AB + N]

            it0 = big.tile([128, NB * IW], I16, tag="it0")
            nc.sync.dma_start(it0[:], idxs)
            # route idx through DVE so the gather carries one DMA-queue wait
            it = big.tile([128, NB * IW], I16, tag="it")
            nc.vector.tensor_copy(it[:], it0[:])

            ones128 = small.tile([128, 1], F32, tag="ones128")
            nc.vector.memset(ones128[:], 1.0)
            onesrow = small.tile([1, 128], F32, tag="onesrow")
            nc.vector.memset(onesrow[:], 1.0)

            # ---------- squared norms ----------
            zsq = big.tile([128, N], F32, tag="zsq")
            nc.vector.tensor_tensor(zsq[:], zT, zT, op=OP.mult)
            zsqown = small.tile([128, R], F32, tag="zsqown")
            nc.vector.tensor_tensor(zsqown[:], zTown, zTown, op=OP.mult)

            n2own_ps = ps_a.tile([1, R], F32, tag="a")
            nc.tensor.matmul(n2own_ps[:], ones128[:], zsqown[:], start=True, stop=True)
            n2own_s = small.tile([1, R], F32, tag="n2own_s")
            nc.vector.tensor_copy(n2own_s[:], n2own_ps[:])
            n2ownrep_ps = ps_a.tile([128, R], F32, tag="a")
            nc.tensor.matmul(n2ownrep_ps[:], onesrow[:], n2own_s[:], start=True, stop=True)
            n2ownrep = small.tile([128, R], F32, tag="n2ownrep")
            nc.vector.tensor_copy(n2ownrep[:], n2ownrep_ps[:])

            n2colc = small.tile([128, CH], F32, tag="n2colc")
            for c in range(CH):
                n2c_ps = ps_a.tile([128, 1], F32, tag="a")
                nc.tensor.matmul(
                    n2c_ps[:], zsq[:, c * 128:(c + 1) * 128], ones128[:],
                    start=True, stop=True,
                )
                nc.vector.tensor_copy(n2colc[:, c:c + 1], n2c_ps[:])

            # ---------- broadcast rows: iota, lo_p, hi_p+1 ----------
            brow_ps = ps_a.tile([128, TW], F32, tag="a")
            nc.tensor.matmul(brow_ps[:], onesrow[:], iotarow, start=True, stop=True)
            trep = big.tile([128, TW], F32, tag="trep")
            nc.vector.tensor_copy(trep[:], brow_ps[:])
            lo_ps = ps_a.tile([128, N], F32, tag="a")
            nc.tensor.matmul(lo_ps[:], onesrow[:], lorow, start=True, stop=True)
            lorep = big.tile([128, N], F32, tag="lorep")
            nc.vector.tensor_copy(lorep[:], lo_ps[:])
            hi_ps = ps_a.tile([128, N], F32, tag="a")
            nc.tensor.matmul(hi_ps[:], onesrow[:], hi1row, start=True, stop=True)
            hi1rep = big.tile([128, N], F32, tag="hi1rep")
            nc.vector.tensor_copy(hi1rep[:], hi_ps[:])

            # Texc[k,t] = [k < t]; TE1[k,p] = [k < lo_p]; TE0[k,p] = [k <= hi_p]
            texc = big.tile([128, CH * TW], BF16, tag="texc")
            te1 = big.tile([128, CH * N], BF16, tag="te1")
            te0 = big.tile([128, CH * N], BF16, tag="te0")
            for c in range(CH):
                jc = jcol[:, c:c + 1]
                nc.vector.tensor_scalar(
                    texc[:, c * TW:(c + 1) * TW], trep[:], jc, None, op0=OP.is_gt
                )
                nc.vector.tensor_scalar(
                    te1[:, c * N:(c + 1) * N], lorep[:], jc, None, op0=OP.is_gt
                )
                nc.vector.tensor_scalar(
                    te0[:, c * N:(c + 1) * N], hi1rep[:], jc, None, op0=OP.is_gt
                )

            # ---------- stage A: w matrix (transposed chunks), phase-batched ----------
            UW = 2 * R
            uvt = big.tile([128, CH * UW], BF16, tag="uvt")
            cs_ps = ps_cs.tile([1, R], F32, tag="cs")
            samet = [chunk.tile([128, R], F32, tag=f"samet{c}", name=f"samet{c}") for c in range(CH)]
            ndt = [chunk.tile([128, R], F32, tag=f"ndt{c}", name=f"ndt{c}") for c in range(CH)]
            sqt = [chunk.tile([128, R], F32, tag=f"sqt{c}", name=f"sqt{c}") for c in range(CH)]
            sqr = [chunk.tile([128, R], F32, tag=f"sqr{c}", name=f"sqr{c}") for c in range(CH)]
            distt = [chunk.tile([128, R], F32, tag=f"distt{c}", name=f"distt{c}") for c in range(CH)]
            et = [chunk.tile([128, R], F32, tag=f"et{c}", name=f"et{c}") for c in range(CH)]
            atcraw = [chunk.tile([128, R], F32, tag=f"atcraw{c}", name=f"atcraw{c}") for c in range(CH)]
            atc = [chunk.tile([128, R], F32, tag=f"atc{c}", name=f"atc{c}") for c in range(CH)]
            dwt = [chunk.tile([128, R], F32, tag=f"dwt{c}", name=f"dwt{c}") for c in range(CH)]
            wt = [chunk.tile([128, R], F32, tag=f"wt{c}", name=f"wt{c}") for c in range(CH)]
            vm = [chunk.tile([128, R], F32, tag=f"vm{c}", name=f"vm{c}") for c in range(CH)]
            wdist = [chunk.tile([128, R], F32, tag=f"wdist{c}", name=f"wdist{c}") for c in range(CH)]
            for c in range(CH):
                ycolbc = yscol[:, c:c + 1].to_broadcast((128, R))
                nc.vector.tensor_tensor(samet[c][:], ysown, ycolbc, op=OP.is_lt)
                nc.vector.tensor_tensor(
                    ndt[c][:], idxown, jcol[:, c:c + 1].to_broadcast((128, R)),
                    op=OP.not_equal,
                )
                nc.vector.tensor_tensor(atcraw[c][:], ysown, ycolbc, op=OP.subtract)
                gt_ps = ps_gt.tile([128, R], F32, tag="gt")
                nc.tensor.matmul(
                    gt_ps[:], zT[:, c * 128:(c + 1) * 128], zTown,
                    start=True, stop=True,
                )
                nc.vector.scalar_tensor_tensor(
                    sqt[c][:], gt_ps[:], -2.0, n2ownrep[:], op0=OP.mult, op1=OP.add
                )
            # batch same-function activations to avoid ACT table reloads
            for c in range(CH):
                nc.scalar.activation(sqr[c][:], sqt[c][:], AF.Relu,
                                     bias=n2colc[:, c:c + 1])
            for c in range(CH):
                nc.scalar.activation(atc[c][:], atcraw[c][:], AF.Abs)
            for c in range(CH):
                nc.scalar.activation(distt[c][:], sqr[c][:], AF.Sqrt)
            for c in range(CH):
                nc.scalar.activation(et[c][:], distt[c][:], AF.Exp, scale=-1.0 / TEMP)
            for c in range(CH):
                nc.scalar.activation(dwt[c][:], atc[c][:], AF.Sigmoid, scale=TAU)
            for c in range(CH):
                nc.vector.tensor_tensor(wt[c][:], et[c][:], dwt[c][:], op=OP.mult)
                nc.vector.tensor_tensor(
                    uvt[:, c * UW:c * UW + R], wt[c][:], samet[c][:], op=OP.mult
                )
                nc.vector.tensor_tensor(vm[c][:], ndt[c][:], samet[c][:], op=OP.subtract)
                nc.vector.tensor_tensor(
                    uvt[:, c * UW + R:(c + 1) * UW], wt[c][:], vm[c][:], op=OP.mult
                )
                nc.vector.tensor_tensor(wdist[c][:], distt[c][:], ndt[c][:], op=OP.mult)
                nc.tensor.matmul(
                    cs_ps[:], ones128[:], wdist[c][:],
                    start=(c == 0), stop=(c == CH - 1),
                )

            # ---------- prefix sums PUex/PVex [48, 385] (cross D table) ----------
            pu_ps = ps_uv.tile([R, TW], F32, tag="pu")
            pv_ps = ps_uv.tile([R, TW], F32, tag="pv")
            for c in range(CH):
                nc.tensor.matmul(
                    pu_ps[:], uvt[:, c * UW:c * UW + R],
                    texc[:, c * TW:(c + 1) * TW],
                    start=(c == 0), stop=(c == CH - 1),
                )
            for c in range(CH):
                nc.tensor.matmul(
                    pv_ps[:], uvt[:, c * UW + R:(c + 1) * UW],
                    texc[:, c * TW:(c + 1) * TW],
                    start=(c == 0), stop=(c == CH - 1),
                )

            # cross D halves (bf16): [(POS_W-1)*PUex | NEG_W*PVex]
            darrA = small.tile([R, TW], BF16, tag="darrA")
            nc.vector.tensor_scalar(darrA[:], pu_ps[:], POS_W - 1.0, None, op0=OP.mult)
            darrB = small.tile([R, TW], BF16, tag="darrB")
            if NEG_W == 1.0:
                nc.vector.tensor_copy(darrB[:], pv_ps[:])
            else:
                nc.vector.tensor_scalar(darrB[:], pv_ps[:], NEG_W, None, op0=OP.mult)
            t1sb = small.tile([R, 1], F32, tag="t1sb")
            nc.vector.tensor_copy(t1sb[:], pu_ps[:, N:N + 1])
            t0sb = small.tile([R, 1], F32, tag="t0sb")
            nc.vector.tensor_copy(t0sb[:], pv_ps[:, N:N + 1])

            # ---------- diag matmuls ----------
            dg1_ps = ps_uv.tile([R, N], F32, tag="pu")
            dg0_ps = ps_uv.tile([R, N], F32, tag="pv")
            for c in range(CH):
                nc.tensor.matmul(
                    dg1_ps[:], uvt[:, c * UW:c * UW + R],
                    te1[:, c * N:(c + 1) * N],
                    start=(c == 0), stop=(c == CH - 1),
                )
            for c in range(CH):
                nc.tensor.matmul(
                    dg0_ps[:], uvt[:, c * UW + R:(c + 1) * UW],
                    te0[:, c * N:(c + 1) * N],
                    start=(c == 0), stop=(c == CH - 1),
                )

            # compact diag row (bf16): dsum = (POS_W-1)*DG1 + T1 + NEG_W*DG0
            # - NEG_W*T0*above  (the above-mask ships from the host)
            dsum_f = small.tile([R, N], F32, tag="dsum_f")
            nc.vector.scalar_tensor_tensor(
                dsum_f[:], dg1_ps[:], POS_W - 1.0,
                t1sb[:].to_broadcast((R, N)), op0=OP.mult, op1=OP.add,
            )
            dsum_g = small.tile([R, N], F32, tag="dsum_g")
            nc.vector.scalar_tensor_tensor(
                dsum_g[:], dg0_ps[:], NEG_W, dsum_f[:], op0=OP.mult, op1=OP.add
            )
            t0neg = small.tile([R, 1], F32, tag="t0neg")
            nc.vector.tensor_scalar(t0neg[:], t0sb[:], -NEG_W, None, op0=OP.mult)
            diag2 = small.tile([R, N], BF16, tag="diag2")
            nc.vector.scalar_tensor_tensor(
                diag2[:], abmask, t0neg[:], dsum_g[:], op0=OP.mult, op1=OP.add
            )

            # lnown = ln(T1 + NEG_W*T0) per row (own-column correction)
            ownden = small.tile([R, 1], F32, tag="ownden")
            nc.vector.scalar_tensor_tensor(
                ownden[:], t0sb[:], NEG_W, t1sb[:], op0=OP.mult, op1=OP.add
            )
            lnown = small.tile([R, 1], F32, tag="lnown")
            nc.scalar.activation(lnown[:], ownden[:], AF.Ln)

            # ---------- replicate x16 via PE, one gather, assemble ----------
            cin = big.tile([128, NB * DW], F32, tag="cin")
            gout = big.tile([128, NB * GW], F32, tag="gout")
            for b in range(NB):
                ebs = pk[0:R, O_EB + b * 64:O_EB + (b + 1) * 64].bitcast(BF16)
                repA_ps = ps_rep.tile([128, TW], F32, tag="rep")
                nc.tensor.matmul(repA_ps[:], ebs, darrA[:], start=True, stop=True)
                repB_ps = ps_rep.tile([128, TW], F32, tag="rep")
                nc.tensor.matmul(repB_ps[:], ebs, darrB[:], start=True, stop=True)
                nc.vector.tensor_copy(cin[:, b * DW:b * DW + TW], repA_ps[:])
                nc.scalar.activation(
                    cin[:, b * DW + TW:(b + 1) * DW], repB_ps[:], AF.Copy
                )
            nc.gpsimd.ap_gather(
                gout[:], cin[:], it[:],
                channels=128, num_elems=NB * DW, d=1, num_idxs=NB * GW,
            )
            denom = big.tile([128, NB * N], F32, tag="denom")
            for b in range(NB):
                ebs = pk[0:R, O_EB + b * 64:O_EB + (b + 1) * 64].bitcast(BF16)
                repD_ps = ps_rep.tile([128, N], F32, tag="rep")
                nc.tensor.matmul(repD_ps[:], ebs, diag2[:], start=True, stop=True)
                nc.vector.tensor_tensor(
                    denom[:, b * N:(b + 1) * N], repD_ps[:],
                    gout[:, b * GW:(b + 1) * GW], op=OP.add,
                )

            lnt = big.tile([128, NB * N], F32, tag="lnt")
            acc = small.tile([128, 1], F32, tag="acc")
            nc.scalar.activation(lnt[:], denom[:], AF.Ln, accum_out=acc[:])

            # ---------- final reduction ----------
            lnacc_ps = ps_a.tile([1, 1], F32, tag="a")
            nc.tensor.matmul(lnacc_ps[:], sel16, acc[:], start=True, stop=True)
            lnacc_s = small.tile([1, 1], F32, tag="lnacc_s")
            nc.vector.tensor_copy(lnacc_s[:], lnacc_ps[:])
            onescol48 = small.tile([R, 1], F32, tag="onescol48")
            nc.vector.memset(onescol48[:], 1.0)
            lnown_ps = ps_a.tile([1, 1], F32, tag="a")
            nc.tensor.matmul(lnown_ps[:], onescol48[:], lnown[:], start=True, stop=True)
            outrow = small.tile([1, R + 2], F32, tag="outrow")
            nc.vector.tensor_copy(outrow[0:1, 0:R], cs_ps[:])
            nc.vector.tensor_copy(outrow[0:1, R:R + 1], lnacc_s[:])
            nc.vector.tensor_copy(outrow[0:1, R + 1:R + 2], lnown_ps[:])
            nc.sync.dma_start(out[0:1, :], outrow[0:1, 0:R])
            nc.sync.dma_start(out[1:2, 0:2], outrow[0:1, R:R + 2])

    nc.compile()
    return nc


_NC_CACHE = None


def _get_nc():
    global _NC_CACHE
    if _NC_CACHE is None:
        _NC_CACHE = _build_program()
    return _NC_CACHE


def _host_prep(embeddings, targets):
    import ml_dtypes

    emb = np.ascontiguousarray(np.asarray(embeddings, dtype=np.float32))
    tgt = np.ascontiguousarray(np.asarray(targets, dtype=np.float32))
    z = emb.transpose(1, 0, 2).reshape(N, D)
    y = np.concatenate([tgt, tgt], axis=0)[:, 0]

    order = np.argsort(y, kind="stable")
    ys = y[order]
    zs = z[order]
    zsT = np.ascontiguousarray(zs.T)  # [D, N]

    # rank tables (depend only on targets)
    A = np.abs(ys[None, :] - ys[:, None]).astype(np.float32)
    hi = np.searchsorted(ys, ys, side="right") - 1
    lo = np.searchsorted(ys, ys, side="left")
    t1 = np.empty((N, N), np.int32)
    t0 = np.empty((N, N), np.int32)
    for m in range(N):
        h = hi[m]
        q1 = np.searchsorted(A[m, h + 1:], A[m], side="left")
        q0 = np.searchsorted(A[m, :h + 1][::-1], A[m], side="left")
        t1[m] = h + 1 + q1
        t0[m] = h + 1 - q0
    above = ys[None, :] > ys[:, None]        # [m, p]

    jidx = np.arange(N, dtype=np.float32)
    iota = np.arange(TW, dtype=np.float32)
    sel = (np.arange(128) % 16 == 0).astype(np.float32)
    ebpack = np.zeros((R, NB * 64), np.float32)
    for bidx in range(NB):
        E = np.zeros((R, 128), np.float32)
        for c in range(128):
            E[8 * bidx + c // 16, c] = 1.0
        ebpack[:, bidx * 64:(bidx + 1) * 64] = (
            E.astype(ml_dtypes.bfloat16).view(np.float32)
        )

    in_maps = []
    for core in range(NC):
        sl = slice(core * R, (core + 1) * R)
        p = np.zeros((128, PW), np.float32)
        p[:, O_ZT:O_ZT + N] = zsT
        p[:, O_ZOWN:O_ZOWN + R] = zsT[:, sl]
        p[:, O_YOWN:O_YOWN + R] = ys[None, sl]
        p[:, O_IOWN:O_IOWN + R] = jidx[None, sl]
        p[:, O_YCOL:O_YCOL + CH] = ys.reshape(CH, 128).T
        p[:, O_JCOL:O_JCOL + CH] = jidx.reshape(CH, 128).T
        p[0, O_IOTA:O_IOTA + TW] = iota
        p[:, O_SEL] = sel
        p[0:R, O_EB:O_EB + NB * 64] = ebpack
        p[0, O_LO:O_LO + N] = lo
        p[0, O_HI1:O_HI1 + N] = hi + 1
        p[0:R, O_AB:O_AB + N] = above[sl].astype(np.float32)

        # cross-gather indices: one per (row, p); group g covers rows {8b+g}
        q = np.empty((8, NB * GW), np.int16)
        for bidx in range(NB):
            rows = core * R + bidx * 8 + np.arange(8)
            off = bidx * DW
            ab = above[rows]
            qb = np.where(ab, TW + t0[rows], t1[rows]) + off
            q[:, bidx * GW:(bidx + 1) * GW] = qb.astype(np.int16)
        it = q.reshape(8, NB * IW, 16).transpose(0, 2, 1).reshape(128, NB * IW)
        in_maps.append({"packed": p, "idxs": it})
    return in_maps


def _reduce_outs(outs_list):
    tot_dist = 0.0
    tot_logd = 0.0
    for o in outs_list:
        o = np.asarray(o, dtype=np.float64)
        tot_dist += o[0, :].sum()
        tot_logd += o[1, 0] - o[1, 1]
    s_total = -tot_dist / TEMP
    loss = -(s_total - tot_logd) / (N * (N - 1))
    return np.float32(loss)


def _run(embeddings, targets, trace=False, **kw):
    nc = _get_nc()
    in_maps = _host_prep(embeddings, targets)
    res = run_bass_kernel_spmd(nc, in_maps, list(range(NC)), trace=trace, **kw)
    outs = [res.results[c]["out"] for c in range(NC)]
    return _reduce_outs(outs), res


def kernel(embeddings, targets):
    loss, _ = _run(embeddings, targets, trace=False)
    return loss
